# revision 64
# baseline (speedup 1.0000x reference)
"""Trainium2 Bass kernel for BlockwiseEarlyExitMamba (~163us, v1 was 222us).

Model: packet embedder -> 4 Mamba blocks (d_model=256, d_inner=512,
d_state=16, dt_rank=16, d_conv=4) -> LayerNorm chain -> early-exit MLP
classifier that reads ONLY position min(32, L)-1 = 31.

Every op in the network is causal, so the [B, 2] output depends only on
x[:, :32, :]; we compute 32 timesteps instead of 1024 (exact).

Sharding: data-parallel over batch, 2 samples/core, weights replicated.

Design notes (what made it fast, in rough order of impact):
 - Few, large weight DMAs: all per-layer weights packed host-side into
   one bf16 blob + one small f32 blob per layer (each dma_start costs
   ~640ns of queue time; v1 spent ~30us just issuing 59 weight DMAs).
   Load order/queue placement chosen so the embedder + layer-0 inputs
   land first; layer-2/3 blobs are issued after the embedder is emitted.
 - in_proj/dt matmuls emit directly in channel-major layout
   ([d partitions, (b t)]) with the weight chunk as the stationary
   operand -- no transposes back from token layout.
   W_dtfull = dt_w @ x_proj_w[:16] is precomputed on host so dt comes
   straight from the conv output; dt_b is pre-filled into PSUM.
 - Everything scan-adjacent lives on the Vector engine: GpSimd shares
   SBUF ports with DVE, and measured concurrency slows BOTH ~2x, so
   offloading there is a mirage.  Instead element counts are minimized:
   dBx, h*C, the n-tree reduction and the gate all run as all-bf16
   tensor_tensors (packed 2-byte operands -> DVE 2x mode).
 - dA = exp(-m*dt), m=1..16: the scalar engine computes m=8,1..7 as
   activations while DVE builds m=9..16 as products dA_8*dA_j in
   parallel (~4us vs 7.8us serial).
 - The B/C state projections are broadcast to all 128 partitions via a
   DRAM round-trip in bf16 with one affine write; B and C are separate
   tiles so the scan's dBx build only waits for the (earlier) B read.
 - The scan itself is 4 tensor_tensor_scan chunks (the hardware scan
   runs ~2.27us per [128, 1024] regardless of dtype; that floor
   dominates the remaining runtime).  h*C + tree round 1 are
   interleaved per chunk behind the scan.
 - On the last layer only t=31 feeds the classifier, so h*C, the tree
   and the gate collapse to one column.
 - LayerNorm uses uncentered stats: sum (DVE) and square-sum (scalar)
   run in parallel in one pass, then a handful of [TOK,1] ops and one
   scalar_tensor_tensor.
"""

import os
import sys

import numpy as np

for _p in ("/root/.axon_site/_ro/trn_rl_repo", "/opt/trn_rl_repo"):
    if os.path.isdir(_p) and _p not in sys.path:
        sys.path.insert(0, _p)

import concourse.bacc as bacc
import concourse.bass as bass
import concourse.mybir as mybir
import concourse.tile as tile
from concourse.bass_utils import run_bass_kernel_spmd

F32 = mybir.dt.float32
BF16 = mybir.dt.bfloat16
AF = mybir.ActivationFunctionType
ALU = mybir.AluOpType

# Pin every activation func this kernel uses to ONE ACT table set, so the
# table-load placement pass emits a single load instead of thrashing.
_ACT_SET = "natural_log_exp_and_others"
_MY_FUNCS = {AF.Exp, AF.Ln, AF.Relu, AF.Square, AF.Identity, AF.Copy}
_orig_get_tables = bacc.get_activation_tables


def _pinned_tables(arch):
    tabs = _orig_get_tables(arch)
    assert _MY_FUNCS <= tabs[_ACT_SET]
    return {name: (funcs if name == _ACT_SET else funcs - _MY_FUNCS)
            for name, funcs in tabs.items()}


bacc.get_activation_tables = _pinned_tables

# Model dims
D_MODEL = 256
D_INNER = 512
D_STATE = 16
D_CONV = 4
DT_RANK = 16
N_LAYERS = 4
BATCH = 16
SEQLEN = 1024
T = 32          # effective timesteps (causal truncation)
N_CORES = 8
B_LOC = BATCH // N_CORES   # 2 samples per core
TOK = B_LOC * T            # 64 tokens per core
NJ = D_INNER // 128        # 4 channel chunks
DM_ROWS = 256 + 1 + 64 + 1 + 2 + 1  # 325 design-matrix rows
SEG = T + 3                # 35: one conv segment incl. 3-col zero gap

# bf16 blob column layout (per layer): [128, WB_COLS]
#   WINT  + (k*8+j)*128 : in_proj stationary chunk; j 0..3 -> x c=j,
#                         4..7 -> z c=j-4; k = d_model chunk
#   WDTF  + (k2*4+c)*128: Wdtf^T chunk, Wdtf = dt_w @ x_proj_w[:16]
#   WOUT  + c*256       : out_proj^T chunk
#   WXBC  + k2*32       : x_proj B/C rows ^T chunk
#   WDP   + c           : D_param (bf16 copy, for the all-bf16 gate)
#   WCW   + (c,b,k)     : conv_w (bf16 copy, for the 2x tap product)
WINT, WDTF, WOUT, WXBC, WDP, WCW = 0, 2048, 4096, 5120, 5248, 5252
WB_COLS = 5284
# f32 blob = smalls [128, 108]:
#   0:32 conv_w (c,b,k), 32:36 conv_b, 36:40 dt_b, 40:104 A, 104:108 D
FB_COLS = 108


def _build_program(a_vals):
    nc = bacc.Bacc(None, target_bir_lowering=False, debug=False)

    # ---------------- DRAM I/O ----------------
    x_d = nc.dram_tensor("x_local", [TOK, 5], F32, kind="ExternalInput")
    embw_d = nc.dram_tensor("embw", [128, 3 * D_MODEL], BF16, kind="ExternalInput")
    wblob_d = nc.dram_tensor("wblob", [N_LAYERS, 128, WB_COLS], BF16,
                             kind="ExternalInput")
    fblob_d = nc.dram_tensor("fblob", [N_LAYERS, 128, FB_COLS], F32,
                             kind="ExternalInput")
    # cls blob: [128, 256 w1t (2x128) | 1 b1 | 2 w2t | 1 b2(rows 0:2)]
    cblob_d = nc.dram_tensor("cblob", [128, 260], F32, kind="ExternalInput")
    out_d = nc.dram_tensor("out", [2, B_LOC], F32, kind="ExternalOutput")

    # B/C scratch, laid out [b][s][n][t] (bf16) so ONE affine write covers
    # both s and the broadcast read is a single contiguous stride-0 DMA of
    # half the bytes.
    bc_scr = nc.dram_tensor("bc_scr", [2 * B_LOC * D_STATE * T], BF16)
    HALF = B_LOC * D_STATE * T  # 1024

    with tile.TileContext(nc) as tc:
        with (
            tc.tile_pool(name="const", bufs=1) as cp,
            tc.tile_pool(name="wpool", bufs=1) as wp,
            tc.tile_pool(name="work", bufs=1) as rp,
            tc.tile_pool(name="scan", bufs=1) as sp,
            tc.tile_pool(name="psmm", bufs=2, space="PSUM") as pmm,
            tc.tile_pool(name="pstr", bufs=2, space="PSUM") as ptr,
            tc.tile_pool(name="psxz", bufs=1, space="PSUM") as pxz,
        ):
            # -------- input + weight DMAs (few, spread over queues) --------
            xq = rp.tile([TOK, 5], F32, name="xq")
            nc.sync.dma_start(xq[:], x_d[:])
            embw_sb = wp.tile([128, 3 * D_MODEL], BF16, name="embw")
            nc.scalar.dma_start(embw_sb[:], embw_d[:])

            # layer 0/1 weights now; layers 2/3 + classifier issued after the
            # embedder is emitted so their queue time cannot delay it.
            # wblob0 is split across two rings so the wint half (needed
            # first, for in_proj) lands ~4us sooner.
            wblob_sb, fblob_sb = [], []
            for l in range(N_LAYERS):
                wblob_sb.append(wp.tile([128, WB_COLS], BF16, name=f"wblob{l}"))
                fblob_sb.append(wp.tile([128, FB_COLS], F32, name=f"fblob{l}"))
            cblob_sb = wp.tile([128, 260], F32, name="cblob")
            nc.sync.dma_start(fblob_sb[0][:], fblob_d[0])
            nc.sync.dma_start(fblob_sb[1][:], fblob_d[1])
            nc.scalar.dma_start(wblob_sb[0][:, :WDTF], wblob_d[0, :, :WDTF])
            nc.scalar.dma_start(wblob_sb[0][:, WDTF:], wblob_d[0, :, WDTF:])
            nc.sync.dma_start(wblob_sb[1][:], wblob_d[1])

            # ---------------- constants ----------------
            ident = cp.tile([128, 128], F32, name="ident")
            nc.gpsimd.memset(ident[:], 0.0)
            nc.gpsimd.affine_select(
                out=ident[:], in_=ident[:], compare_op=ALU.not_equal,
                fill=1.0, base=0, pattern=[[-1, 128]], channel_multiplier=1)
            iota257 = cp.tile([TOK, 257], F32, name="iota257")
            nc.gpsimd.iota(iota257[:], pattern=[[1, 257]], base=0,
                           channel_multiplier=0,
                           allow_small_or_imprecise_dtypes=True)
            eps_t = cp.tile([128, 1], F32, name="eps_t")
            nc.vector.memset(eps_t[:], 1e-5)

            # ---------------- embedder ----------------
            # One-hot of int(clip(x)) as a difference of >= comparisons.
            dm = rp.tile([TOK, DM_ROWS], F32, name="dm")
            ge_p = rp.tile([TOK, 257], F32, name="ge_p")
            nc.vector.tensor_tensor(
                ge_p[:], xq[:, 0:1].broadcast_to([TOK, 257]), iota257[:],
                op=ALU.is_ge)
            nc.vector.tensor_sub(dm[:, 0:256], ge_p[:, 0:256], ge_p[:, 1:257])
            ge_f = rp.tile([TOK, 65], F32, name="ge_f")
            nc.vector.tensor_tensor(
                ge_f[:], xq[:, 2:3].broadcast_to([TOK, 65]), iota257[:, 0:65],
                op=ALU.is_ge)
            nc.vector.tensor_sub(dm[:, 257:321], ge_f[:, 0:64], ge_f[:, 1:65])
            ge_d = rp.tile([TOK, 3], F32, name="ge_d")
            nc.vector.tensor_tensor(
                ge_d[:], xq[:, 4:5].broadcast_to([TOK, 3]), iota257[:, 0:3],
                op=ALU.is_ge)
            nc.vector.tensor_sub(dm[:, 322:324], ge_d[:, 0:2], ge_d[:, 1:3])
            dmcols = bass.AP(dm[:].tensor, dm[:, 256].offset,
                             [dm[:].ap[0], [65, 2]])
            xqcols = bass.AP(xq[:].tensor, xq[:, 1].offset,
                             [xq[:].ap[0], [2, 2]])
            nc.scalar.copy(dmcols, xqcols)
            nc.vector.memset(dm[:, 324:325], 1.0)

            feat_ps = pmm.tile([TOK, D_MODEL], F32, name="feat_ps", tag="mm")
            for c, (r0, r1) in enumerate(((0, 128), (128, 256), (256, DM_ROWS))):
                w = r1 - r0
                tp = ptr.tile([128, TOK], F32, name=f"dmt_ps{c}", tag="tr")
                nc.tensor.transpose(tp[:w, :], dm[:, r0:r1], ident[:TOK, :TOK])
                dmt = rp.tile([128, TOK], BF16, name=f"dmt{c}", tag="dmt")
                nc.scalar.copy(dmt[:w, :], tp[:w, :])
                nc.tensor.matmul(feat_ps[:], dmt[:w, :],
                                 embw_sb[:w, c * D_MODEL:(c + 1) * D_MODEL],
                                 start=(c == 0), stop=(c == 2))

            def layer_norm(src_ap, dst):
                """dst = LN(src) over free dim (256), no affine (g=1, b=0).

                Uses uncentered stats (var = E[x^2] - m^2) so the sum
                (DVE) and square-sum (scalar) run in parallel on one pass
                over src, then a handful of [TOK,1] ops and one STT.
                """
                nsum = rp.tile([TOK, 1], F32, name="nsum", tag="lnstat")
                nc.vector.tensor_reduce(nsum[:], src_ap, axis=mybir.AxisListType.X,
                                        op=ALU.add)
                sq = rp.tile([TOK, D_MODEL], F32, name="sq", tag="lnsq")
                vsum = rp.tile([TOK, 1], F32, name="vsum", tag="lnstat3")
                nc.scalar.activation(sq[:], src_ap, AF.Square, accum_out=vsum[:])
                m = rp.tile([TOK, 1], F32, name="lnm", tag="lnstat2")
                nc.vector.tensor_scalar_mul(m[:], nsum[:], 1.0 / D_MODEL)
                q = rp.tile([TOK, 1], F32, name="lnq", tag="lnstat6")
                nc.scalar.activation(q[:], nsum[:], AF.Square,
                                     scale=1.0 / D_MODEL)
                u = rp.tile([TOK, 1], F32, name="lnu", tag="lnstat7")
                nc.vector.scalar_tensor_tensor(
                    u[:], vsum[:], 1.0 / D_MODEL, q[:],
                    op0=ALU.mult, op1=ALU.subtract)
                lnv = rp.tile([TOK, 1], F32, name="lnv", tag="lnstat4")
                nc.scalar.activation(lnv[:], u[:], AF.Ln, bias=eps_t[:TOK, :])
                rstd = rp.tile([TOK, 1], F32, name="rstd", tag="lnstat5")
                nc.scalar.activation(rstd[:], lnv[:], AF.Exp, scale=-0.5)
                rstd_b = bass.AP(rstd[:].tensor, rstd[:].offset,
                                 [rstd[:].ap[0], [0, D_MODEL]])
                nc.vector.scalar_tensor_tensor(
                    dst, src_ap, m[:], rstd_b,
                    op0=ALU.subtract, op1=ALU.mult)

            feat = rp.tile([TOK, D_MODEL], F32, name="feat_init")
            layer_norm(feat_ps[:], feat[:])

            # late weight loads (layers 2/3, classifier) -- issued here so
            # their DMA-queue time sits behind the embedder, not before it
            nc.scalar.dma_start(wblob_sb[2][:], wblob_d[2])
            nc.scalar.dma_start(wblob_sb[3][:], wblob_d[3])
            nc.sync.dma_start(fblob_sb[2][:], fblob_d[2])
            nc.sync.dma_start(fblob_sb[3][:], fblob_d[3])
            nc.sync.dma_start(cblob_sb[:], cblob_d[:])

            # ---------------- Mamba layers ----------------
            # conv scratch with zero gaps (zeroed once, stays zero)
            xpad = rp.tile([128, NJ * B_LOC * SEG], BF16, name="xpad")
            gaps = bass.AP(xpad[:].tensor, xpad[:].offset,
                           [xpad[:].ap[0], [SEG, NJ * B_LOC], [1, 3]])
            nc.vector.memset(gaps, 0.0)
            # dA scratch (bufs=1 slot reused each layer): t=0 of every
            # (c,b,n) segment must read 0 so the scan resets per segment;
            # nothing ever writes those columns, so zero them ONCE here.
            scna = sp.tile([128, NJ, B_LOC, D_STATE, T], BF16, name="scna")
            t0 = bass.AP(scna[:].tensor, scna[:].offset,
                         [scna[:].ap[0], [B_LOC * D_STATE * T, NJ],
                          [T, B_LOC * D_STATE], [1, 1]])
            if a_vals is not None:
                nc.vector.memset(t0, 0.0)
            for l in range(N_LAYERS):
                wb = wblob_sb[l]
                fb = fblob_sb[l]

                # featT [256, TOK] as two 128-row chunks, bf16
                featT = rp.tile([128, 2 * TOK], BF16, name=f"featT{l}",
                                tag="featT")
                for c in range(2):
                    tp = ptr.tile([128, TOK], F32, name=f"ftp{l}_{c}", tag="tr")
                    nc.tensor.transpose(tp[:], feat[:, c * 128:(c + 1) * 128],
                                        ident[:TOK, :TOK])
                    nc.scalar.copy(featT[:, c * TOK:(c + 1) * TOK], tp[:])

                # in_proj directly into channel-major layout:
                # xz[j-chunk, (b t)] in PSUM; j 0..3 -> x c=j, 4..7 -> z.
                # x and z halves in separate PSUM tiles so the conv copy can
                # start as soon as the 4 x-chunks are done.
                xz_ps = pxz.tile([128, 4 * TOK], F32, name=f"xz{l}", tag="xz")
                z_ps = pxz.tile([128, 4 * TOK], F32, name=f"z{l}", tag="z")
                for j in range(8):  # x chunks first
                    dst = (xz_ps if j < 4 else z_ps)
                    jj = j % 4
                    for k in range(2):
                        nc.tensor.matmul(
                            dst[:, jj * TOK:(jj + 1) * TOK],
                            wb[:, WINT + (k * 8 + j) * 128:
                               WINT + (k * 8 + j + 1) * 128],
                            featT[:, k * TOK:(k + 1) * TOK],
                            start=(k == 0), stop=(k == 1))

                # conv: one wide PSUM->zero-gap-SBUF copy, then tap-product
                # + tap-reduce + bias add.
                cpsrc = bass.AP(xz_ps[:].tensor, xz_ps[:].offset,
                                [xz_ps[:].ap[0], [T, NJ * B_LOC], [1, T]])
                cpdst = bass.AP(xpad[:].tensor, xpad[:, 3].offset,
                                [xpad[:].ap[0], [SEG, NJ * B_LOC], [1, T]])
                nc.scalar.copy(cpdst, cpsrc)
                cprod = rp.tile([128, NJ * B_LOC, T, D_CONV], BF16,
                                name=f"cprod{l}", tag="cprod")
                in0 = bass.AP(xpad[:].tensor, xpad[:].offset,
                              [xpad[:].ap[0], [SEG, NJ * B_LOC], [1, T],
                               [1, D_CONV]])
                in1 = bass.AP(wb[:].tensor, wb[:, WCW].offset,
                              [wb[:].ap[0], [D_CONV, NJ * B_LOC], [0, T],
                               [1, D_CONV]])
                nc.vector.tensor_tensor(cprod[:], in0, in1, op=ALU.mult)
                vpre = rp.tile([128, NJ, B_LOC, T], F32, name=f"vpre{l}",
                               tag="vpre")
                nc.vector.tensor_reduce(
                    vpre[:].rearrange("p a b t -> p (a b) t"), cprod[:],
                    axis=mybir.AxisListType.X, op=ALU.add)
                cb_ap = bass.AP(fb[:].tensor, fb[:, 32].offset,
                                [fb[:].ap[0], [1, NJ], [0, B_LOC], [0, T]])
                nc.vector.tensor_add(vpre[:], vpre[:], cb_ap)

                # silu(v) = v * sigmoid(v); sigmoid via exp/ln chain.
                # xcall comes out in bf16 (it is a matmul operand below).
                vflat = vpre[:].rearrange("p a b t -> p (a b t)")
                sg = rp.tile([128, NJ * B_LOC * T], F32, name=f"sg{l}", tag="sg")
                nc.scalar.activation(sg[:], vflat, AF.Exp, scale=-1.0)
                nc.scalar.activation(sg[:], sg[:], AF.Ln, bias=1.0)
                nc.scalar.activation(sg[:], sg[:], AF.Exp, scale=-1.0)
                xcall = rp.tile([128, NJ, B_LOC, T], BF16, name=f"xcall{l}",
                                tag="xcall")
                nc.vector.tensor_mul(
                    xcall[:].rearrange("p a b t -> p (a b t)"), vflat, sg[:])

                # dt_b pre-fill of the dtpre PSUM accumulator (c varies,
                # broadcast over (b t)); matmuls below use start=False.
                dtpre_ps = pmm.tile([128, NJ * TOK], F32, name=f"dtpre{l}",
                                    tag="mm")
                dtb_src = bass.AP(fb[:].tensor, fb[:, 36].offset,
                                  [fb[:].ap[0], [1, NJ], [0, TOK]])
                dtb_dst = bass.AP(dtpre_ps[:].tensor, dtpre_ps[:].offset,
                                  [dtpre_ps[:].ap[0], [TOK, NJ], [1, TOK]])
                nc.vector.tensor_scalar_add(dtb_dst, dtb_src, 0.0)

                # x_proj B/C rows + dt_pre, straight from xcall chunks.
                dbl_ps = ptr.tile([2 * D_STATE, TOK], F32, name=f"dbl{l}",
                                  tag="tr")
                for k2 in range(NJ):
                    nc.tensor.matmul(
                        dbl_ps[:],
                        wb[:, WXBC + k2 * 32:WXBC + (k2 + 1) * 32],
                        xcall[:, k2].rearrange("p b t -> p (b t)"),
                        start=(k2 == 0), stop=(k2 == NJ - 1))
                for c in range(NJ):
                    for k2 in range(NJ):
                        nc.tensor.matmul(
                            dtpre_ps[:, c * TOK:(c + 1) * TOK],
                            wb[:, WDTF + (k2 * 4 + c) * 128:
                               WDTF + (k2 * 4 + c + 1) * 128],
                            xcall[:, k2].rearrange("p b t -> p (b t)"),
                            start=False, stop=(k2 == NJ - 1),
                            skip_group_check=True)

                # B/C -> DRAM [b][s][n][t] (bf16) with ONE affine write
                # (row r = s*16+n maps to offset 32*r), then ONE stride-0
                # broadcast read across all 128 partitions.
                dbl_sb = rp.tile([2 * D_STATE, TOK], BF16, name=f"dblsb{l}",
                                 tag="dblsb")
                nc.scalar.copy(dbl_sb[:], dbl_ps[:])
                dst = bass.AP(bc_scr[:].tensor, 0,
                              [[T, 2 * D_STATE], [2 * D_STATE * T, B_LOC],
                               [1, T]])
                nc.sync.dma_start(dst, dbl_sb[:])
                # separate B and C tiles: scnb only has to wait for the
                # (earlier) B read.  Layout per tile: [b][n][t].
                NT = D_STATE * T
                brep = rp.tile([128, HALF], BF16, name=f"brep{l}", tag="brep")
                nc.sync.dma_start(
                    brep[:],
                    bass.AP(bc_scr[:].tensor, 0,
                            [[0, 128], [2 * NT, B_LOC], [1, NT]]))
                crep = rp.tile([128, HALF], BF16, name=f"crep{l}", tag="crep")
                nc.sync.dma_start(
                    crep[:],
                    bass.AP(bc_scr[:].tensor, NT,
                            [[0, 128], [2 * NT, B_LOC], [1, NT]]))

                # softplus(dtpre) = ln(1 + exp(dtpre)) -- 2 wide ACTs
                # (bias is already in the PSUM accumulator)
                dtall = rp.tile([128, NJ, B_LOC, T], F32, name=f"dtall{l}",
                                tag="dtall")
                dtflat = dtall[:].rearrange("p a b t -> p (a b t)")
                nc.scalar.activation(dtflat, dtpre_ps[:], AF.Exp, scale=1.0)
                nc.scalar.activation(dtflat, dtflat, AF.Ln, bias=1.0)

                # dtx = dt * xc (bf16 so scnb below runs in DVE 2x mode)
                dtx = rp.tile([128, NJ, B_LOC, T], BF16, name=f"dtx{l}",
                              tag="dtx")
                nc.vector.tensor_mul(
                    dtx[:].rearrange("p a b t -> p (a b t)"), dtflat,
                    xcall[:].rearrange("p a b t -> p (a b t)"))

                # dA = exp(dt * A); t=0 columns are pre-zeroed (see above).
                # a_vals path: A[:, n] = a_{n} is d-independent, so
                # dA_m = exp(-m*dt) for m = n+1 in 1..16.  The scalar engine
                # computes m = 8, 1..7 as activations; DVE builds m = 9..16
                # as dA_8 * dA_{m-8} in parallel with the scalar chain.
                def dA_slice(n):
                    return bass.AP(
                        scna[:].tensor, scna[:, 0, 0, n, 1].offset,
                        [scna[:].ap[0], [B_LOC * NT, NJ],
                         [NT, B_LOC], [1, T - 1]])

                if a_vals is not None:
                    src = bass.AP(
                        dtall[:].tensor, dtall[:, 0, 0, 1].offset,
                        [dtall[:].ap[0], [B_LOC * T, NJ], [T, B_LOC],
                         [1, T - 1]])
                    # exact powers only when a_vals has the -(n+1) structure
                    # fp32 exp(log(m)) roundtrip leaves ~1e-6 relative error;
                    # the product structure only needs a_{8+j} ~= a_8 + a_j,
                    # which holds to ~1e-6 relative -> harmless in dA.
                    pow_ok = all(abs(a_vals[l][n] + (n + 1)) < 1e-3 * (n + 1)
                                 for n in range(D_STATE))
                    if pow_ok:
                        for n in (7, 0, 1, 2, 3, 4, 5, 6):
                            nc.scalar.activation(dA_slice(n), src, AF.Exp,
                                                 scale=float(a_vals[l][n]))
                        for j in range(8):  # dA_{9+j-1}: n = 8..15
                            nc.vector.tensor_tensor(
                                dA_slice(8 + j), dA_slice(7), dA_slice(j),
                                op=ALU.mult)
                    else:
                        for n in range(D_STATE):
                            nc.scalar.activation(dA_slice(n), src, AF.Exp,
                                                 scale=float(a_vals[l][n]))
                else:
                    for c in range(NJ):
                        in0g = bass.AP(
                            dtall[:].tensor, dtall[:, c, 0, 0].offset,
                            [dtall[:].ap[0], [T, B_LOC], [0, D_STATE], [1, T]])
                        in1g = bass.AP(
                            fb[:].tensor, fb[:, 40 + c * D_STATE].offset,
                            [fb[:].ap[0], [0, B_LOC], [1, D_STATE], [0, T]])
                        nc.vector.tensor_tensor(
                            scna[:, c], in0g, in1g, op=ALU.mult)
                    body = bass.AP(
                        scna[:].tensor, scna[:, 0, 0, 0, 1].offset,
                        [scna[:].ap[0], [T, NJ * B_LOC * D_STATE], [1, T - 1]])
                    nc.scalar.activation(body, body, AF.Exp)
                    t0g = bass.AP(scna[:].tensor, scna[:].offset,
                                  [scna[:].ap[0], [B_LOC * D_STATE * T, NJ],
                                   [T, B_LOC * D_STATE], [1, 1]])
                    nc.vector.memset(t0g, 0.0)

                # THE scan (DVE).  scnb (dt*x*B) and h*C_rep stay on DVE too
                # (GpSimd would fight DVE for SBUF ports and slow both ~2x);
                # both run in bf16 (all operands 2-byte -> DVE 2x mode),
                # interleaved chunk-by-chunk with the scan.
                scnb = sp.tile([128, NJ, B_LOC, D_STATE, T], BF16,
                               name=f"scnb{l}", tag="scnb")
                hh = sp.tile([128, NJ, B_LOC, D_STATE, T], BF16,
                             name=f"hh{l}", tag="hh")
                hc = sp.tile([128, NJ, B_LOC, D_STATE, T], BF16,
                             name=f"hc{l}", tag="hc")

                def scnb_c(c):
                    in0c = bass.AP(dtx[:].tensor, dtx[:, c, 0, 0].offset,
                                   [dtx[:].ap[0], [T, B_LOC], [0, D_STATE],
                                    [1, T]])
                    in1c = bass.AP(brep[:].tensor, brep[:].offset,
                                   [brep[:].ap[0], [NT, B_LOC], [T, D_STATE],
                                    [1, T]])
                    outc = bass.AP(scnb[:].tensor, scnb[:, c, 0, 0, 0].offset,
                                   [scnb[:].ap[0], [NT, B_LOC], [T, D_STATE],
                                    [1, T]])
                    nc.vector.tensor_tensor(outc, in0c, in1c, op=ALU.mult)

                # On the last layer only token t = T-1 feeds the output
                # (classifier reads feat[:, 31]), so the C-projection, the
                # n-tree and the gate shrink to that single column; stale
                # values in the other columns are finite and never read.
                toff, tcnt = (T - 1, 1) if l == N_LAYERS - 1 else (0, T)
                scnb_c(0)
                for c in range(NJ):
                    nc.vector.tensor_tensor_scan(
                        hh[:, c].rearrange("p b n t -> p (b n t)"),
                        scna[:, c].rearrange("p b n t -> p (b n t)"),
                        scnb[:, c].rearrange("p b n t -> p (b n t)"),
                        initial=0.0, op0=ALU.mult, op1=ALU.add)
                    if c + 1 < NJ:
                        scnb_c(c + 1)
                    cr_ap = bass.AP(crep[:].tensor, crep[:, toff].offset,
                                    [crep[:].ap[0], [NT, B_LOC],
                                     [T, D_STATE], [1, tcnt]])
                    hh_c = bass.AP(hh[:].tensor, hh[:, c, 0, 0, toff].offset,
                                   [hh[:].ap[0], [NT, B_LOC], [T, D_STATE],
                                    [1, tcnt]])
                    hc_c = bass.AP(hc[:].tensor, hc[:, c, 0, 0, toff].offset,
                                   [hc[:].ap[0], [NT, B_LOC], [T, D_STATE],
                                    [1, tcnt]])
                    nc.vector.tensor_tensor(hc_c, hh_c, cr_ap, op=ALU.mult)
                    # tree round 1 for this chunk, pipelined behind its hc
                    lo1 = bass.AP(hc[:].tensor, hc[:, c, 0, 0, toff].offset,
                                  [hc[:].ap[0], [NT, B_LOC], [T, 8],
                                   [1, tcnt]])
                    hi1 = bass.AP(hc[:].tensor, hc[:, c, 0, 8, toff].offset,
                                  [hc[:].ap[0], [NT, B_LOC], [T, 8],
                                   [1, tcnt]])
                    nc.vector.tensor_add(lo1, lo1, hi1)

                # remaining n-tree rounds across all chunks (all-bf16 2x);
                # result lands in the n=0 slice.
                h = 8
                while h > 1:
                    h //= 2
                    lo = bass.AP(hc[:].tensor, hc[:, 0, 0, 0, toff].offset,
                                 [hc[:].ap[0], [NT, NJ * B_LOC], [T, h],
                                  [1, tcnt]])
                    hi = bass.AP(hc[:].tensor, hc[:, 0, 0, h, toff].offset,
                                 [hc[:].ap[0], [NT, NJ * B_LOC], [T, h],
                                  [1, tcnt]])
                    nc.vector.tensor_add(lo, lo, hi)
                ys_ap = bass.AP(hc[:].tensor, hc[:, 0, 0, 0, toff].offset,
                                [hc[:].ap[0], [NT, NJ * B_LOC], [1, tcnt]])

                # sigmoid gate z*sigmoid(z), sigmoid via exp/ln chain,
                # straight off the z PSUM; bf16 result for the 2x gate TTs.
                zraw = bass.AP(z_ps[:].tensor, z_ps[:].offset,
                               [z_ps[:].ap[0], [1, NJ * B_LOC * T]])
                zsig = rp.tile([128, NJ * B_LOC * T], F32, name=f"zsig{l}",
                               tag="zsig")
                nc.scalar.activation(zsig[:], zraw, AF.Exp, scale=-1.0)
                nc.scalar.activation(zsig[:], zsig[:], AF.Ln, bias=1.0)
                nc.scalar.activation(zsig[:], zsig[:], AF.Exp, scale=-1.0)
                zsigb = rp.tile([128, NJ * B_LOC * T], BF16, name=f"zsb{l}",
                                tag="zsb")
                nc.vector.tensor_mul(zsigb[:], zsig[:], zraw)

                # y = (ys + D * xc) * z * sigmoid(z) -- all-bf16 TTs (2x)
                yg = rp.tile([128, NJ, B_LOC, T], BF16, name=f"yg{l}", tag="yg")
                d_ap = bass.AP(wb[:].tensor, wb[:, WDP].offset,
                               [wb[:].ap[0], [1, NJ], [0, B_LOC], [0, tcnt]])
                yg_s = bass.AP(yg[:].tensor, yg[:, 0, 0, toff].offset,
                               [yg[:].ap[0], [B_LOC * T, NJ], [T, B_LOC],
                                [1, tcnt]])
                xc_s = bass.AP(xcall[:].tensor, xcall[:, 0, 0, toff].offset,
                               [xcall[:].ap[0], [B_LOC * T, NJ], [T, B_LOC],
                                [1, tcnt]])
                nc.vector.tensor_tensor(yg_s, xc_s, d_ap, op=ALU.mult)
                ygf = bass.AP(yg[:].tensor, yg[:, 0, 0, toff].offset,
                              [yg[:].ap[0], [T, NJ * B_LOC], [1, tcnt]])
                nc.vector.tensor_add(ygf, ygf, ys_ap)
                ygr = rp.tile([128, NJ, B_LOC, T], BF16, name=f"ygr{l}",
                              tag="ygr")
                ygr_s = bass.AP(ygr[:].tensor, ygr[:, 0, 0, toff].offset,
                                [ygr[:].ap[0], [T, NJ * B_LOC], [1, tcnt]])
                zs_s = bass.AP(zsigb[:].tensor, zsigb[:, toff].offset,
                               [zsigb[:].ap[0], [T, NJ * B_LOC], [1, tcnt]])
                nc.vector.tensor_tensor(ygr_s, ygf, zs_s, op=ALU.mult)

                # out_proj + residual + LN
                yout_ps = pmm.tile([TOK, D_MODEL], F32, name=f"yout{l}",
                                   tag="mm2")
                for c in range(NJ):
                    nc.tensor.matmul(
                        yout_ps[:], ygr[:, c].rearrange("p b t -> p (b t)"),
                        wb[:, WOUT + c * D_MODEL:WOUT + (c + 1) * D_MODEL],
                        start=(c == 0), stop=(c == NJ - 1))
                fsum = rp.tile([TOK, D_MODEL], F32, name=f"fsum{l}", tag="fsum")
                nc.vector.tensor_add(fsum[:], yout_ps[:], feat[:])
                feat = rp.tile([TOK, D_MODEL], F32, name=f"feat{l}",
                               tag="featv2")
                layer_norm(fsum[:], feat[:])

            # ---------------- classifier (token t=31 per sample) ----------
            cls_in = rp.tile([B_LOC, D_MODEL], F32, name="cls_in")
            for b in range(B_LOC):
                r = b * T + (T - 1)
                nc.sync.dma_start(cls_in[b:b + 1, :], feat[r:r + 1, :])
            clsT = rp.tile([128, 2 * B_LOC], F32, name="clsT")
            for c in range(2):
                tp = ptr.tile([128, B_LOC], F32, name=f"clsT_ps{c}", tag="tr")
                nc.tensor.transpose(tp[:], cls_in[:, c * 128:(c + 1) * 128],
                                    ident[:B_LOC, :B_LOC])
                nc.scalar.copy(clsT[:, c * B_LOC:(c + 1) * B_LOC], tp[:])
            q1_ps = pmm.tile([128, B_LOC], F32, name="q1_ps", tag="mm")
            for c in range(2):
                nc.tensor.matmul(q1_ps[:], cblob_sb[:, c * 128:(c + 1) * 128],
                                 clsT[:, c * B_LOC:(c + 1) * B_LOC],
                                 start=(c == 0), stop=(c == 1))
            r1 = rp.tile([128, B_LOC], F32, name="r1")
            nc.scalar.activation(r1[:], q1_ps[:], AF.Relu,
                                 bias=cblob_sb[:, 256:257], scale=1.0)
            o_ps = pmm.tile([2, B_LOC], F32, name="o_ps", tag="mm2")
            nc.tensor.matmul(o_ps[:], cblob_sb[:, 257:259], r1[:],
                             start=True, stop=True)
            out_sb = rp.tile([2, B_LOC], F32, name="out_sb")
            nc.scalar.activation(out_sb[:], o_ps[:], AF.Identity,
                                 bias=cblob_sb[0:2, 259:260], scale=1.0)
            nc.sync.dma_start(out_d[:], out_sb[:])

    nc.finalize()
    return nc


def _prep_host(inputs):
    """Host-side weight preprocessing (pure reshaping/merging, exact math)."""
    import ml_dtypes

    g = lambda k: np.asarray(inputs[k], dtype=np.float32)

    fusion_w = g("fusion_w")          # [256, 136]
    wf_proto = fusion_w[:, 0:32]
    wf_len = fusion_w[:, 32:64]
    wf_flags = fusion_w[:, 64:96]
    wf_iat = fusion_w[:, 96:128]
    wf_dir = fusion_w[:, 128:136]

    embw = np.zeros((DM_ROWS, D_MODEL), np.float32)
    embw[0:256] = g("emb_proto") @ wf_proto.T
    embw[256] = wf_len @ g("proj_len_w")[:, 0]
    embw[257:321] = g("emb_flags") @ wf_flags.T
    embw[321] = wf_iat @ g("proj_iat_w")[:, 0]
    embw[322:324] = g("emb_dir") @ wf_dir.T
    embw[324] = (g("fusion_b") + wf_len @ g("proj_len_b")
                 + wf_iat @ g("proj_iat_b"))
    import ml_dtypes
    embw_p = np.zeros((128, 3 * D_MODEL), ml_dtypes.bfloat16)
    for c, (r0, r1) in enumerate(((0, 128), (128, 256), (256, DM_ROWS))):
        embw_p[:r1 - r0, c * D_MODEL:(c + 1) * D_MODEL] = embw[r0:r1]

    A = -np.exp(g("A_log"))           # [L, 512, 16]
    if bool(np.all(A == A[:, :1, :])):
        a_vals = tuple(tuple(float(v) for v in A[l, 0]) for l in range(N_LAYERS))
    else:
        a_vals = None

    wblob = np.zeros((N_LAYERS, 128, WB_COLS), ml_dtypes.bfloat16)
    fblob = np.zeros((N_LAYERS, 128, FB_COLS), np.float32)
    for l in range(N_LAYERS):
        wint = g("in_proj_w")[l].T            # [256, 1024]
        for k in range(2):
            for j in range(8):
                wblob[l, :, WINT + (k * 8 + j) * 128:
                      WINT + (k * 8 + j + 1) * 128] = \
                    wint[k * 128:(k + 1) * 128, j * 128:(j + 1) * 128]
        wdtf = (g("dt_w")[l] @ g("x_proj_w")[l][:DT_RANK, :]).T  # [din, dout]
        for k2 in range(NJ):
            for c in range(NJ):
                wblob[l, :, WDTF + (k2 * 4 + c) * 128:
                      WDTF + (k2 * 4 + c + 1) * 128] = \
                    wdtf[k2 * 128:(k2 + 1) * 128, c * 128:(c + 1) * 128]
        wout = g("out_proj_w")[l].T           # [512, 256]
        for c in range(NJ):
            wblob[l, :, WOUT + c * D_MODEL:WOUT + (c + 1) * D_MODEL] = \
                wout[c * 128:(c + 1) * 128]
        wxbc = g("x_proj_w")[l][DT_RANK:, :].T  # [512, 32]
        for k2 in range(NJ):
            wblob[l, :, WXBC + k2 * 32:WXBC + (k2 + 1) * 32] = \
                wxbc[k2 * 128:(k2 + 1) * 128]
        wblob[l, :, WDP:WDP + NJ] = g("D_param")[l].reshape(NJ, 128).T
        cw_b = np.transpose(g("conv_w")[l].reshape(NJ, 128, D_CONV), (1, 0, 2))
        wblob[l, :, WCW:WCW + 32] = np.repeat(cw_b, B_LOC, axis=1).reshape(128, 32)

        cw = g("conv_w")[l].reshape(NJ, 128, D_CONV)          # [j, p, k]
        cwp = np.transpose(cw, (1, 0, 2))                     # [p, j, k]
        fblob[l, :, 0:32] = np.repeat(cwp, B_LOC, axis=1).reshape(128, 32)
        fblob[l, :, 32:36] = g("conv_b")[l].reshape(NJ, 128).T
        fblob[l, :, 36:40] = g("dt_b")[l].reshape(NJ, 128).T
        Aj = A[l].reshape(NJ, 128, D_STATE)                   # [j, p, n]
        fblob[l, :, 40:104] = np.transpose(Aj, (1, 0, 2)).reshape(128, 64)
        fblob[l, :, 104:108] = g("D_param")[l].reshape(NJ, 128).T

    cblob = np.zeros((128, 260), np.float32)
    w1t = g("cls_w1").T                       # [256, 128]
    cblob[:, 0:128] = w1t[0:128]
    cblob[:, 128:256] = w1t[128:256]
    cblob[:, 256] = g("cls_b1")
    cblob[:, 257:259] = g("cls_w2").T
    cblob[0:2, 259] = g("cls_b2")

    common = {
        "embw": embw_p, "wblob": wblob, "fblob": fblob, "cblob": cblob,
    }

    x = g("x")[:, :T, :]              # causal truncation: only 32 steps matter
    in_maps = []
    for i in range(N_CORES):
        m = dict(common)
        m["x_local"] = np.ascontiguousarray(
            x[i * B_LOC:(i + 1) * B_LOC].reshape(TOK, 5))
        in_maps.append(m)
    return in_maps, a_vals


_PROGRAM_CACHE = {}


def kernel(**inputs) -> np.ndarray:
    in_maps, a_vals = _prep_host(inputs)
    nc = _PROGRAM_CACHE.get(a_vals)
    if nc is None:
        nc = _build_program(a_vals)
        _PROGRAM_CACHE[a_vals] = nc
    res = run_bass_kernel_spmd(nc, in_maps, core_ids=list(range(N_CORES)))
    out = np.zeros((BATCH, 2), np.float32)
    for i in range(N_CORES):
        out[i * B_LOC:(i + 1) * B_LOC] = np.asarray(res.results[i]["out"]).T
    return out


# revision 65
# speedup vs baseline: 1.0095x; 1.0095x over previous
"""Trainium2 Bass kernel for BlockwiseEarlyExitMamba (~163us; v1 was 222us).

Model: packet embedder -> 4 Mamba blocks (d_model=256, d_inner=512,
d_state=16, dt_rank=16, d_conv=4) -> LayerNorm chain -> early-exit MLP
classifier that reads ONLY position min(32, L)-1 = 31.

Every op in the network is causal, so the [B, 2] output depends only on
x[:, :32, :]; we compute 32 timesteps instead of 1024 (exact).

Sharding: data-parallel over batch, 2 samples/core, weights replicated.

Design notes (what made it fast, in rough order of impact):
 - Few, large weight DMAs: all per-layer weights packed host-side into
   one bf16 blob + one small f32 blob per layer (each dma_start costs
   ~640ns of queue time; v1 spent ~30us just issuing 59 weight DMAs).
   Load order/queue placement chosen so the embedder + layer-0 inputs
   land first; layer-2/3 blobs are issued after the embedder is emitted.
 - in_proj/dt matmuls emit directly in channel-major layout
   ([d partitions, (b t)]) with the weight chunk as the stationary
   operand -- no transposes back from token layout.
   W_dtfull = dt_w @ x_proj_w[:16] is precomputed on host so dt comes
   straight from the conv output; dt_b is pre-filled into PSUM.
 - Everything scan-adjacent lives on the Vector engine: GpSimd shares
   SBUF ports with DVE, and measured concurrency slows BOTH ~2x, so
   offloading there is a mirage.  Instead element counts are minimized:
   dBx, h*C, the n-tree reduction and the gate all run as all-bf16
   tensor_tensors (packed 2-byte operands -> DVE 2x mode).
 - dA = exp(-m*dt), m=1..16: the scalar engine computes m=8,1..7 as
   activations while DVE builds m=9..16 as products dA_8*dA_j in
   parallel (~4us vs 7.8us serial).
 - The B/C state projections are broadcast to all 128 partitions via a
   DRAM round-trip in bf16 with one affine write; B and C are separate
   tiles so the scan's dBx build only waits for the (earlier) B read.
 - The scan itself is 4 tensor_tensor_scan chunks (the hardware scan
   runs ~2.27us per [128, 1024] regardless of dtype; that floor
   dominates the remaining runtime).  h*C + tree round 1 are
   interleaved per chunk behind the scan.
 - On the last layer only t=31 feeds the classifier, so h*C, the tree
   and the gate collapse to one column.
 - LayerNorm uses uncentered stats: sum (DVE) and square-sum (scalar)
   run in parallel in one pass, then a handful of [TOK,1] ops and one
   scalar_tensor_tensor.
"""

import os
import sys

import numpy as np

for _p in ("/root/.axon_site/_ro/trn_rl_repo", "/opt/trn_rl_repo"):
    if os.path.isdir(_p) and _p not in sys.path:
        sys.path.insert(0, _p)

import concourse.bacc as bacc
import concourse.bass as bass
import concourse.mybir as mybir
import concourse.tile as tile
from concourse.bass_utils import run_bass_kernel_spmd

F32 = mybir.dt.float32
BF16 = mybir.dt.bfloat16
AF = mybir.ActivationFunctionType
ALU = mybir.AluOpType

# Pin every activation func this kernel uses to ONE ACT table set, so the
# table-load placement pass emits a single load instead of thrashing.
_ACT_SET = "natural_log_exp_and_others"
_MY_FUNCS = {AF.Exp, AF.Ln, AF.Relu, AF.Square, AF.Identity, AF.Copy}
_orig_get_tables = bacc.get_activation_tables


def _pinned_tables(arch):
    tabs = _orig_get_tables(arch)
    assert _MY_FUNCS <= tabs[_ACT_SET]
    return {name: (funcs if name == _ACT_SET else funcs - _MY_FUNCS)
            for name, funcs in tabs.items()}


bacc.get_activation_tables = _pinned_tables

# Model dims
D_MODEL = 256
D_INNER = 512
D_STATE = 16
D_CONV = 4
DT_RANK = 16
N_LAYERS = 4
BATCH = 16
SEQLEN = 1024
T = 32          # effective timesteps (causal truncation)
N_CORES = 8
B_LOC = BATCH // N_CORES   # 2 samples per core
TOK = B_LOC * T            # 64 tokens per core
NJ = D_INNER // 128        # 4 channel chunks
DM_ROWS = 256 + 1 + 64 + 1 + 2 + 1  # 325 design-matrix rows
SEG = T + 3                # 35: one conv segment incl. 3-col zero gap

# bf16 blob column layout (per layer): [128, WB_COLS]
#   WINT  + (k*8+j)*128 : in_proj stationary chunk; j 0..3 -> x c=j,
#                         4..7 -> z c=j-4; k = d_model chunk
#   WDTF  + (k2*4+c)*128: Wdtf^T chunk, Wdtf = dt_w @ x_proj_w[:16]
#   WOUT  + c*256       : out_proj^T chunk
#   WXBC  + k2*32       : x_proj B/C rows ^T chunk
#   WDP   + c           : D_param (bf16 copy, for the all-bf16 gate)
#   WCW   + (c,b,k)     : conv_w (bf16 copy, for the 2x tap product)
WINT, WDTF, WOUT, WXBC, WDP, WCW = 0, 2048, 4096, 5120, 5248, 5252
WB_COLS = 5284
# f32 blob = smalls [128, 108]:
#   0:32 conv_w (c,b,k), 32:36 conv_b, 36:40 dt_b, 40:104 A, 104:108 D
FB_COLS = 108


def _build_program(a_vals):
    nc = bacc.Bacc(None, target_bir_lowering=False, debug=False)

    # ---------------- DRAM I/O ----------------
    x_d = nc.dram_tensor("x_local", [TOK, 5], F32, kind="ExternalInput")
    embw_d = nc.dram_tensor("embw", [128, 3 * D_MODEL], BF16, kind="ExternalInput")
    wblob_d = nc.dram_tensor("wblob", [N_LAYERS, 128, WB_COLS], BF16,
                             kind="ExternalInput")
    fblob_d = nc.dram_tensor("fblob", [N_LAYERS, 128, FB_COLS], F32,
                             kind="ExternalInput")
    # cls blob: [128, 256 w1t (2x128) | 1 b1 | 2 w2t | 1 b2(rows 0:2)]
    cblob_d = nc.dram_tensor("cblob", [128, 260], F32, kind="ExternalInput")
    out_d = nc.dram_tensor("out", [2, B_LOC], F32, kind="ExternalOutput")

    # B/C scratch, laid out [b][s][n][t] (bf16) so ONE affine write covers
    # both s and the broadcast read is a single contiguous stride-0 DMA of
    # half the bytes.
    bc_scr = nc.dram_tensor("bc_scr", [2 * B_LOC * D_STATE * T], BF16)
    HALF = B_LOC * D_STATE * T  # 1024

    with tile.TileContext(nc) as tc:
        with (
            tc.tile_pool(name="const", bufs=1) as cp,
            tc.tile_pool(name="wpool", bufs=1) as wp,
            tc.tile_pool(name="work", bufs=1) as rp,
            tc.tile_pool(name="scan", bufs=1) as sp,
            tc.tile_pool(name="psmm", bufs=2, space="PSUM") as pmm,
            tc.tile_pool(name="pstr", bufs=2, space="PSUM") as ptr,
            tc.tile_pool(name="psxz", bufs=1, space="PSUM") as pxz,
        ):
            # -------- input + weight DMAs (few, spread over queues) --------
            xq = rp.tile([TOK, 5], F32, name="xq")
            nc.sync.dma_start(xq[:], x_d[:])
            embw_sb = wp.tile([128, 3 * D_MODEL], BF16, name="embw")
            nc.scalar.dma_start(embw_sb[:], embw_d[:])

            # layer 0/1 weights now; layers 2/3 + classifier issued after the
            # embedder is emitted so their queue time cannot delay it.
            # wblob0 is split across two rings so the wint half (needed
            # first, for in_proj) lands ~4us sooner.
            wblob_sb, fblob_sb = [], []
            for l in range(N_LAYERS):
                wblob_sb.append(wp.tile([128, WB_COLS], BF16, name=f"wblob{l}"))
                fblob_sb.append(wp.tile([128, FB_COLS], F32, name=f"fblob{l}"))
            cblob_sb = wp.tile([128, 260], F32, name="cblob")
            nc.sync.dma_start(fblob_sb[0][:], fblob_d[0])
            nc.sync.dma_start(fblob_sb[1][:], fblob_d[1])
            nc.scalar.dma_start(wblob_sb[0][:, :WDTF], wblob_d[0, :, :WDTF])
            nc.scalar.dma_start(wblob_sb[0][:, WDTF:], wblob_d[0, :, WDTF:])
            nc.sync.dma_start(wblob_sb[1][:], wblob_d[1])

            # ---------------- constants ----------------
            ident = cp.tile([128, 128], F32, name="ident")
            nc.gpsimd.memset(ident[:], 0.0)
            nc.gpsimd.affine_select(
                out=ident[:], in_=ident[:], compare_op=ALU.not_equal,
                fill=1.0, base=0, pattern=[[-1, 128]], channel_multiplier=1)
            iota257 = cp.tile([TOK, 257], F32, name="iota257")
            nc.gpsimd.iota(iota257[:], pattern=[[1, 257]], base=0,
                           channel_multiplier=0,
                           allow_small_or_imprecise_dtypes=True)
            eps_t = cp.tile([128, 1], F32, name="eps_t")
            nc.vector.memset(eps_t[:], 1e-5)

            # ---------------- embedder ----------------
            # One-hot of int(clip(x)) as a difference of >= comparisons.
            dm = rp.tile([TOK, DM_ROWS], F32, name="dm")
            ge_p = rp.tile([TOK, 257], F32, name="ge_p")
            nc.vector.tensor_tensor(
                ge_p[:], xq[:, 0:1].broadcast_to([TOK, 257]), iota257[:],
                op=ALU.is_ge)
            nc.vector.tensor_sub(dm[:, 0:256], ge_p[:, 0:256], ge_p[:, 1:257])
            ge_f = rp.tile([TOK, 65], F32, name="ge_f")
            nc.vector.tensor_tensor(
                ge_f[:], xq[:, 2:3].broadcast_to([TOK, 65]), iota257[:, 0:65],
                op=ALU.is_ge)
            nc.vector.tensor_sub(dm[:, 257:321], ge_f[:, 0:64], ge_f[:, 1:65])
            ge_d = rp.tile([TOK, 3], F32, name="ge_d")
            nc.vector.tensor_tensor(
                ge_d[:], xq[:, 4:5].broadcast_to([TOK, 3]), iota257[:, 0:3],
                op=ALU.is_ge)
            nc.vector.tensor_sub(dm[:, 322:324], ge_d[:, 0:2], ge_d[:, 1:3])
            dmcols = bass.AP(dm[:].tensor, dm[:, 256].offset,
                             [dm[:].ap[0], [65, 2]])
            xqcols = bass.AP(xq[:].tensor, xq[:, 1].offset,
                             [xq[:].ap[0], [2, 2]])
            nc.scalar.copy(dmcols, xqcols)
            nc.vector.memset(dm[:, 324:325], 1.0)

            feat_ps = pmm.tile([TOK, D_MODEL], F32, name="feat_ps", tag="mm")
            for c, (r0, r1) in enumerate(((0, 128), (128, 256), (256, DM_ROWS))):
                w = r1 - r0
                tp = ptr.tile([128, TOK], F32, name=f"dmt_ps{c}", tag="tr")
                nc.tensor.transpose(tp[:w, :], dm[:, r0:r1], ident[:TOK, :TOK])
                dmt = rp.tile([128, TOK], BF16, name=f"dmt{c}", tag="dmt")
                nc.scalar.copy(dmt[:w, :], tp[:w, :])
                nc.tensor.matmul(feat_ps[:], dmt[:w, :],
                                 embw_sb[:w, c * D_MODEL:(c + 1) * D_MODEL],
                                 start=(c == 0), stop=(c == 2))

            def layer_norm(src_ap, dst):
                """dst = LN(src) over free dim (256), no affine (g=1, b=0).

                Uses uncentered stats (var = E[x^2] - m^2) so the sum
                (DVE) and square-sum (scalar) run in parallel on one pass
                over src, then a handful of [TOK,1] ops and one STT.
                """
                nsum = rp.tile([TOK, 1], F32, name="nsum", tag="lnstat")
                nc.vector.tensor_reduce(nsum[:], src_ap, axis=mybir.AxisListType.X,
                                        op=ALU.add)
                sq = rp.tile([TOK, D_MODEL], F32, name="sq", tag="lnsq")
                vsum = rp.tile([TOK, 1], F32, name="vsum", tag="lnstat3")
                nc.scalar.activation(sq[:], src_ap, AF.Square, accum_out=vsum[:])
                m = rp.tile([TOK, 1], F32, name="lnm", tag="lnstat2")
                nc.vector.tensor_scalar_mul(m[:], nsum[:], 1.0 / D_MODEL)
                q = rp.tile([TOK, 1], F32, name="lnq", tag="lnstat6")
                nc.scalar.activation(q[:], nsum[:], AF.Square,
                                     scale=1.0 / D_MODEL)
                u = rp.tile([TOK, 1], F32, name="lnu", tag="lnstat7")
                nc.vector.scalar_tensor_tensor(
                    u[:], vsum[:], 1.0 / D_MODEL, q[:],
                    op0=ALU.mult, op1=ALU.subtract)
                lnv = rp.tile([TOK, 1], F32, name="lnv", tag="lnstat4")
                nc.scalar.activation(lnv[:], u[:], AF.Ln, bias=eps_t[:TOK, :])
                rstd = rp.tile([TOK, 1], F32, name="rstd", tag="lnstat5")
                nc.scalar.activation(rstd[:], lnv[:], AF.Exp, scale=-0.5)
                rstd_b = bass.AP(rstd[:].tensor, rstd[:].offset,
                                 [rstd[:].ap[0], [0, D_MODEL]])
                nc.vector.scalar_tensor_tensor(
                    dst, src_ap, m[:], rstd_b,
                    op0=ALU.subtract, op1=ALU.mult)

            feat = rp.tile([TOK, D_MODEL], F32, name="feat_init")
            layer_norm(feat_ps[:], feat[:])

            # late weight loads (layers 2/3, classifier) -- issued here so
            # their DMA-queue time sits behind the embedder, not before it
            nc.scalar.dma_start(wblob_sb[2][:], wblob_d[2])
            nc.scalar.dma_start(wblob_sb[3][:], wblob_d[3])
            nc.sync.dma_start(fblob_sb[2][:], fblob_d[2])
            nc.sync.dma_start(fblob_sb[3][:], fblob_d[3])
            nc.sync.dma_start(cblob_sb[:], cblob_d[:])

            # ---------------- Mamba layers ----------------
            # conv scratch with zero gaps (zeroed once, stays zero)
            xpad = rp.tile([128, NJ * B_LOC * SEG], BF16, name="xpad")
            gaps = bass.AP(xpad[:].tensor, xpad[:].offset,
                           [xpad[:].ap[0], [SEG, NJ * B_LOC], [1, 3]])
            nc.vector.memset(gaps, 0.0)
            # dA scratch (bufs=1 slot reused each layer): t=0 of every
            # (c,b,n) segment must read 0 so the scan resets per segment;
            # nothing ever writes those columns, so zero them ONCE here.
            scna = sp.tile([128, NJ, B_LOC, D_STATE, T], BF16, name="scna")
            t0 = bass.AP(scna[:].tensor, scna[:].offset,
                         [scna[:].ap[0], [B_LOC * D_STATE * T, NJ],
                          [T, B_LOC * D_STATE], [1, 1]])
            if a_vals is not None:
                nc.vector.memset(t0, 0.0)
            for l in range(N_LAYERS):
                wb = wblob_sb[l]
                fb = fblob_sb[l]

                # featT [256, TOK] as two 128-row chunks, bf16
                featT = rp.tile([128, 2 * TOK], BF16, name=f"featT{l}",
                                tag="featT")
                for c in range(2):
                    tp = ptr.tile([128, TOK], F32, name=f"ftp{l}_{c}", tag="tr")
                    nc.tensor.transpose(tp[:], feat[:, c * 128:(c + 1) * 128],
                                        ident[:TOK, :TOK])
                    nc.scalar.copy(featT[:, c * TOK:(c + 1) * TOK], tp[:])

                # in_proj directly into channel-major layout:
                # xz[j-chunk, (b t)] in PSUM; j 0..3 -> x c=j, 4..7 -> z.
                # x and z halves in separate PSUM tiles so the conv copy can
                # start as soon as the 4 x-chunks are done.
                xz_ps = pxz.tile([128, 4 * TOK], F32, name=f"xz{l}", tag="xz")
                z_ps = pxz.tile([128, 4 * TOK], F32, name=f"z{l}", tag="z")
                for j in range(8):  # x chunks first
                    dst = (xz_ps if j < 4 else z_ps)
                    jj = j % 4
                    for k in range(2):
                        nc.tensor.matmul(
                            dst[:, jj * TOK:(jj + 1) * TOK],
                            wb[:, WINT + (k * 8 + j) * 128:
                               WINT + (k * 8 + j + 1) * 128],
                            featT[:, k * TOK:(k + 1) * TOK],
                            start=(k == 0), stop=(k == 1))

                # conv: one wide PSUM->zero-gap-SBUF copy, then tap-product
                # + tap-reduce + bias add.
                cpsrc = bass.AP(xz_ps[:].tensor, xz_ps[:].offset,
                                [xz_ps[:].ap[0], [T, NJ * B_LOC], [1, T]])
                cpdst = bass.AP(xpad[:].tensor, xpad[:, 3].offset,
                                [xpad[:].ap[0], [SEG, NJ * B_LOC], [1, T]])
                nc.scalar.copy(cpdst, cpsrc)
                cprod = rp.tile([128, NJ * B_LOC, T, D_CONV], BF16,
                                name=f"cprod{l}", tag="cprod")
                in0 = bass.AP(xpad[:].tensor, xpad[:].offset,
                              [xpad[:].ap[0], [SEG, NJ * B_LOC], [1, T],
                               [1, D_CONV]])
                in1 = bass.AP(wb[:].tensor, wb[:, WCW].offset,
                              [wb[:].ap[0], [D_CONV, NJ * B_LOC], [0, T],
                               [1, D_CONV]])
                nc.vector.tensor_tensor(cprod[:], in0, in1, op=ALU.mult)
                vpre = rp.tile([128, NJ, B_LOC, T], F32, name=f"vpre{l}",
                               tag="vpre")
                nc.vector.tensor_reduce(
                    vpre[:].rearrange("p a b t -> p (a b) t"), cprod[:],
                    axis=mybir.AxisListType.X, op=ALU.add)
                cb_ap = bass.AP(fb[:].tensor, fb[:, 32].offset,
                                [fb[:].ap[0], [1, NJ], [0, B_LOC], [0, T]])
                nc.vector.tensor_add(vpre[:], vpre[:], cb_ap)

                # silu(v) = v * sigmoid(v); sigmoid via exp/ln chain.
                # xcall comes out in bf16 (it is a matmul operand below).
                vflat = vpre[:].rearrange("p a b t -> p (a b t)")
                sg = rp.tile([128, NJ * B_LOC * T], F32, name=f"sg{l}", tag="sg")
                nc.scalar.activation(sg[:], vflat, AF.Exp, scale=-1.0)
                nc.scalar.activation(sg[:], sg[:], AF.Ln, bias=1.0)
                nc.scalar.activation(sg[:], sg[:], AF.Exp, scale=-1.0)
                xcall = rp.tile([128, NJ, B_LOC, T], BF16, name=f"xcall{l}",
                                tag="xcall")
                nc.vector.tensor_mul(
                    xcall[:].rearrange("p a b t -> p (a b t)"), vflat, sg[:])

                # dt_b pre-fill of the dtpre PSUM accumulator (c varies,
                # broadcast over (b t)); matmuls below use start=False.
                dtpre_ps = pmm.tile([128, NJ * TOK], F32, name=f"dtpre{l}",
                                    tag="mm")
                dtb_src = bass.AP(fb[:].tensor, fb[:, 36].offset,
                                  [fb[:].ap[0], [1, NJ], [0, TOK]])
                dtb_dst = bass.AP(dtpre_ps[:].tensor, dtpre_ps[:].offset,
                                  [dtpre_ps[:].ap[0], [TOK, NJ], [1, TOK]])
                nc.vector.tensor_scalar_add(dtb_dst, dtb_src, 0.0)

                # x_proj B/C rows + dt_pre, straight from xcall chunks.
                dbl_ps = ptr.tile([2 * D_STATE, TOK], F32, name=f"dbl{l}",
                                  tag="tr")
                for k2 in range(NJ):
                    nc.tensor.matmul(
                        dbl_ps[:],
                        wb[:, WXBC + k2 * 32:WXBC + (k2 + 1) * 32],
                        xcall[:, k2].rearrange("p b t -> p (b t)"),
                        start=(k2 == 0), stop=(k2 == NJ - 1))
                for c in range(NJ):
                    for k2 in range(NJ):
                        nc.tensor.matmul(
                            dtpre_ps[:, c * TOK:(c + 1) * TOK],
                            wb[:, WDTF + (k2 * 4 + c) * 128:
                               WDTF + (k2 * 4 + c + 1) * 128],
                            xcall[:, k2].rearrange("p b t -> p (b t)"),
                            start=False, stop=(k2 == NJ - 1),
                            skip_group_check=True)

                # B/C -> DRAM [b][s][n][t] (bf16) with ONE affine write
                # (row r = s*16+n maps to offset 32*r), then ONE stride-0
                # broadcast read across all 128 partitions.
                dbl_sb = rp.tile([2 * D_STATE, TOK], BF16, name=f"dblsb{l}",
                                 tag="dblsb")
                nc.scalar.copy(dbl_sb[:], dbl_ps[:])
                dst = bass.AP(bc_scr[:].tensor, 0,
                              [[T, 2 * D_STATE], [2 * D_STATE * T, B_LOC],
                               [1, T]])
                nc.sync.dma_start(dst, dbl_sb[:])
                # separate B and C tiles: scnb only has to wait for the
                # (earlier) B read.  Layout per tile: [b][n][t].
                NT = D_STATE * T
                brep = rp.tile([128, HALF], BF16, name=f"brep{l}", tag="brep")
                nc.sync.dma_start(
                    brep[:],
                    bass.AP(bc_scr[:].tensor, 0,
                            [[0, 128], [2 * NT, B_LOC], [1, NT]]))
                crep = rp.tile([128, HALF], BF16, name=f"crep{l}", tag="crep")
                nc.sync.dma_start(
                    crep[:],
                    bass.AP(bc_scr[:].tensor, NT,
                            [[0, 128], [2 * NT, B_LOC], [1, NT]]))

                # softplus(dtpre) = ln(1 + exp(dtpre)) -- 2 wide ACTs
                # (bias is already in the PSUM accumulator)
                dtall = rp.tile([128, NJ, B_LOC, T], F32, name=f"dtall{l}",
                                tag="dtall")
                dtflat = dtall[:].rearrange("p a b t -> p (a b t)")
                nc.scalar.activation(dtflat, dtpre_ps[:], AF.Exp, scale=1.0)
                nc.scalar.activation(dtflat, dtflat, AF.Ln, bias=1.0)

                # dtx = dt * xc (bf16 so scnb below runs in DVE 2x mode)
                dtx = rp.tile([128, NJ, B_LOC, T], BF16, name=f"dtx{l}",
                              tag="dtx")
                nc.vector.tensor_mul(
                    dtx[:].rearrange("p a b t -> p (a b t)"), dtflat,
                    xcall[:].rearrange("p a b t -> p (a b t)"))

                # dA = exp(dt * A); t=0 columns are pre-zeroed (see above).
                # a_vals path: A[:, n] = a_{n} is d-independent, so
                # dA_m = exp(-m*dt) for m = n+1 in 1..16.  The scalar engine
                # computes m = 8, 1..7 as activations; DVE builds m = 9..16
                # as dA_8 * dA_{m-8} in parallel with the scalar chain.
                def dA_slice(n):
                    return bass.AP(
                        scna[:].tensor, scna[:, 0, 0, n, 1].offset,
                        [scna[:].ap[0], [B_LOC * NT, NJ],
                         [NT, B_LOC], [1, T - 1]])

                if a_vals is not None:
                    src = bass.AP(
                        dtall[:].tensor, dtall[:, 0, 0, 1].offset,
                        [dtall[:].ap[0], [B_LOC * T, NJ], [T, B_LOC],
                         [1, T - 1]])
                    # exact powers only when a_vals has the -(n+1) structure
                    # fp32 exp(log(m)) roundtrip leaves ~1e-6 relative error;
                    # the product structure only needs a_{8+j} ~= a_8 + a_j,
                    # which holds to ~1e-6 relative -> harmless in dA.
                    pow_ok = all(abs(a_vals[l][n] + (n + 1)) < 1e-3 * (n + 1)
                                 for n in range(D_STATE))
                    if pow_ok:
                        for n in (7, 0, 1, 2, 3, 4, 5, 6):
                            nc.scalar.activation(dA_slice(n), src, AF.Exp,
                                                 scale=float(a_vals[l][n]))
                        for j in range(8):  # dA_{9+j-1}: n = 8..15
                            nc.vector.tensor_tensor(
                                dA_slice(8 + j), dA_slice(7), dA_slice(j),
                                op=ALU.mult)
                    else:
                        for n in range(D_STATE):
                            nc.scalar.activation(dA_slice(n), src, AF.Exp,
                                                 scale=float(a_vals[l][n]))
                else:
                    for c in range(NJ):
                        in0g = bass.AP(
                            dtall[:].tensor, dtall[:, c, 0, 0].offset,
                            [dtall[:].ap[0], [T, B_LOC], [0, D_STATE], [1, T]])
                        in1g = bass.AP(
                            fb[:].tensor, fb[:, 40 + c * D_STATE].offset,
                            [fb[:].ap[0], [0, B_LOC], [1, D_STATE], [0, T]])
                        nc.vector.tensor_tensor(
                            scna[:, c], in0g, in1g, op=ALU.mult)
                    body = bass.AP(
                        scna[:].tensor, scna[:, 0, 0, 0, 1].offset,
                        [scna[:].ap[0], [T, NJ * B_LOC * D_STATE], [1, T - 1]])
                    nc.scalar.activation(body, body, AF.Exp)
                    t0g = bass.AP(scna[:].tensor, scna[:].offset,
                                  [scna[:].ap[0], [B_LOC * D_STATE * T, NJ],
                                   [T, B_LOC * D_STATE], [1, 1]])
                    nc.vector.memset(t0g, 0.0)

                # THE scan (DVE).  scnb (dt*x*B) and h*C_rep stay on DVE too
                # (GpSimd would fight DVE for SBUF ports and slow both ~2x);
                # both run in bf16 (all operands 2-byte -> DVE 2x mode),
                # interleaved chunk-by-chunk with the scan.
                scnb = sp.tile([128, NJ, B_LOC, D_STATE, T], BF16,
                               name=f"scnb{l}", tag="scnb")
                hh = sp.tile([128, NJ, B_LOC, D_STATE, T], BF16,
                             name=f"hh{l}", tag="hh")
                hc = sp.tile([128, NJ, B_LOC, D_STATE, T], BF16,
                             name=f"hc{l}", tag="hc")

                def scnb_c(c):
                    in0c = bass.AP(dtx[:].tensor, dtx[:, c, 0, 0].offset,
                                   [dtx[:].ap[0], [T, B_LOC], [0, D_STATE],
                                    [1, T]])
                    in1c = bass.AP(brep[:].tensor, brep[:].offset,
                                   [brep[:].ap[0], [NT, B_LOC], [T, D_STATE],
                                    [1, T]])
                    outc = bass.AP(scnb[:].tensor, scnb[:, c, 0, 0, 0].offset,
                                   [scnb[:].ap[0], [NT, B_LOC], [T, D_STATE],
                                    [1, T]])
                    nc.vector.tensor_tensor(outc, in0c, in1c, op=ALU.mult)

                # On the last layer only token t = T-1 feeds the output
                # (classifier reads feat[:, 31]), so the C-projection, the
                # n-tree and the gate shrink to that single column; stale
                # values in the other columns are finite and never read.
                toff, tcnt = (T - 1, 1) if l == N_LAYERS - 1 else (0, T)
                scnb_c(0)
                for c in range(NJ):
                    nc.vector.tensor_tensor_scan(
                        hh[:, c].rearrange("p b n t -> p (b n t)"),
                        scna[:, c].rearrange("p b n t -> p (b n t)"),
                        scnb[:, c].rearrange("p b n t -> p (b n t)"),
                        initial=0.0, op0=ALU.mult, op1=ALU.add)
                    if c + 1 < NJ:
                        scnb_c(c + 1)
                    cr_ap = bass.AP(crep[:].tensor, crep[:, toff].offset,
                                    [crep[:].ap[0], [NT, B_LOC],
                                     [T, D_STATE], [1, tcnt]])
                    hh_c = bass.AP(hh[:].tensor, hh[:, c, 0, 0, toff].offset,
                                   [hh[:].ap[0], [NT, B_LOC], [T, D_STATE],
                                    [1, tcnt]])
                    hc_c = bass.AP(hc[:].tensor, hc[:, c, 0, 0, toff].offset,
                                   [hc[:].ap[0], [NT, B_LOC], [T, D_STATE],
                                    [1, tcnt]])
                    nc.vector.tensor_tensor(hc_c, hh_c, cr_ap, op=ALU.mult)
                    # tree round 1 for this chunk, pipelined behind its hc
                    lo1 = bass.AP(hc[:].tensor, hc[:, c, 0, 0, toff].offset,
                                  [hc[:].ap[0], [NT, B_LOC], [T, 8],
                                   [1, tcnt]])
                    hi1 = bass.AP(hc[:].tensor, hc[:, c, 0, 8, toff].offset,
                                  [hc[:].ap[0], [NT, B_LOC], [T, 8],
                                   [1, tcnt]])
                    nc.vector.tensor_add(lo1, lo1, hi1)

                # remaining n-tree rounds across all chunks (all-bf16 2x);
                # result lands in the n=0 slice.
                h = 8
                while h > 1:
                    h //= 2
                    lo = bass.AP(hc[:].tensor, hc[:, 0, 0, 0, toff].offset,
                                 [hc[:].ap[0], [NT, NJ * B_LOC], [T, h],
                                  [1, tcnt]])
                    hi = bass.AP(hc[:].tensor, hc[:, 0, 0, h, toff].offset,
                                 [hc[:].ap[0], [NT, NJ * B_LOC], [T, h],
                                  [1, tcnt]])
                    nc.vector.tensor_add(lo, lo, hi)
                ys_ap = bass.AP(hc[:].tensor, hc[:, 0, 0, 0, toff].offset,
                                [hc[:].ap[0], [NT, NJ * B_LOC], [1, tcnt]])

                # sigmoid gate z*sigmoid(z), sigmoid via exp/ln chain,
                # straight off the z PSUM; bf16 result for the 2x gate TTs.
                zraw = bass.AP(z_ps[:].tensor, z_ps[:].offset,
                               [z_ps[:].ap[0], [1, NJ * B_LOC * T]])
                zsig = rp.tile([128, NJ * B_LOC * T], F32, name=f"zsig{l}",
                               tag="zsig")
                nc.scalar.activation(zsig[:], zraw, AF.Exp, scale=-1.0)
                nc.scalar.activation(zsig[:], zsig[:], AF.Ln, bias=1.0)
                nc.scalar.activation(zsig[:], zsig[:], AF.Exp, scale=-1.0)
                zsigb = rp.tile([128, NJ * B_LOC * T], BF16, name=f"zsb{l}",
                                tag="zsb")
                nc.vector.tensor_mul(zsigb[:], zsig[:], zraw)

                # y = (ys + D * xc) * z * sigmoid(z) -- all-bf16 TTs (2x)
                yg = rp.tile([128, NJ, B_LOC, T], BF16, name=f"yg{l}", tag="yg")
                d_ap = bass.AP(wb[:].tensor, wb[:, WDP].offset,
                               [wb[:].ap[0], [1, NJ], [0, B_LOC], [0, tcnt]])
                yg_s = bass.AP(yg[:].tensor, yg[:, 0, 0, toff].offset,
                               [yg[:].ap[0], [B_LOC * T, NJ], [T, B_LOC],
                                [1, tcnt]])
                xc_s = bass.AP(xcall[:].tensor, xcall[:, 0, 0, toff].offset,
                               [xcall[:].ap[0], [B_LOC * T, NJ], [T, B_LOC],
                                [1, tcnt]])
                nc.vector.tensor_tensor(yg_s, xc_s, d_ap, op=ALU.mult)
                ygf = bass.AP(yg[:].tensor, yg[:, 0, 0, toff].offset,
                              [yg[:].ap[0], [T, NJ * B_LOC], [1, tcnt]])
                nc.vector.tensor_add(ygf, ygf, ys_ap)
                ygr = rp.tile([128, NJ, B_LOC, T], BF16, name=f"ygr{l}",
                              tag="ygr")
                ygr_s = bass.AP(ygr[:].tensor, ygr[:, 0, 0, toff].offset,
                                [ygr[:].ap[0], [T, NJ * B_LOC], [1, tcnt]])
                zs_s = bass.AP(zsigb[:].tensor, zsigb[:, toff].offset,
                               [zsigb[:].ap[0], [T, NJ * B_LOC], [1, tcnt]])
                nc.vector.tensor_tensor(ygr_s, ygf, zs_s, op=ALU.mult)

                # out_proj + residual + LN
                yout_ps = pmm.tile([TOK, D_MODEL], F32, name=f"yout{l}",
                                   tag="mm2")
                for c in range(NJ):
                    nc.tensor.matmul(
                        yout_ps[:], ygr[:, c].rearrange("p b t -> p (b t)"),
                        wb[:, WOUT + c * D_MODEL:WOUT + (c + 1) * D_MODEL],
                        start=(c == 0), stop=(c == NJ - 1))
                fsum = rp.tile([TOK, D_MODEL], F32, name=f"fsum{l}", tag="fsum")
                nc.vector.tensor_add(fsum[:], yout_ps[:], feat[:])
                feat = rp.tile([TOK, D_MODEL], F32, name=f"feat{l}",
                               tag="featv2")
                layer_norm(fsum[:], feat[:])

            # ---------------- classifier (token t=31 per sample) ----------
            cls_in = rp.tile([B_LOC, D_MODEL], F32, name="cls_in")
            for b in range(B_LOC):
                r = b * T + (T - 1)
                nc.sync.dma_start(cls_in[b:b + 1, :], feat[r:r + 1, :])
            clsT = rp.tile([128, 2 * B_LOC], F32, name="clsT")
            for c in range(2):
                tp = ptr.tile([128, B_LOC], F32, name=f"clsT_ps{c}", tag="tr")
                nc.tensor.transpose(tp[:], cls_in[:, c * 128:(c + 1) * 128],
                                    ident[:B_LOC, :B_LOC])
                nc.scalar.copy(clsT[:, c * B_LOC:(c + 1) * B_LOC], tp[:])
            q1_ps = pmm.tile([128, B_LOC], F32, name="q1_ps", tag="mm")
            for c in range(2):
                nc.tensor.matmul(q1_ps[:], cblob_sb[:, c * 128:(c + 1) * 128],
                                 clsT[:, c * B_LOC:(c + 1) * B_LOC],
                                 start=(c == 0), stop=(c == 1))
            r1 = rp.tile([128, B_LOC], F32, name="r1")
            nc.scalar.activation(r1[:], q1_ps[:], AF.Relu,
                                 bias=cblob_sb[:, 256:257], scale=1.0)
            o_ps = pmm.tile([2, B_LOC], F32, name="o_ps", tag="mm2")
            nc.tensor.matmul(o_ps[:], cblob_sb[:, 257:259], r1[:],
                             start=True, stop=True)
            out_sb = rp.tile([2, B_LOC], F32, name="out_sb")
            nc.scalar.activation(out_sb[:], o_ps[:], AF.Identity,
                                 bias=cblob_sb[0:2, 259:260], scale=1.0)
            nc.sync.dma_start(out_d[:], out_sb[:])

    nc.finalize()
    return nc


def _prep_host(inputs):
    """Host-side weight preprocessing (pure reshaping/merging, exact math)."""
    import ml_dtypes

    g = lambda k: np.asarray(inputs[k], dtype=np.float32)

    fusion_w = g("fusion_w")          # [256, 136]
    wf_proto = fusion_w[:, 0:32]
    wf_len = fusion_w[:, 32:64]
    wf_flags = fusion_w[:, 64:96]
    wf_iat = fusion_w[:, 96:128]
    wf_dir = fusion_w[:, 128:136]

    embw = np.zeros((DM_ROWS, D_MODEL), np.float32)
    embw[0:256] = g("emb_proto") @ wf_proto.T
    embw[256] = wf_len @ g("proj_len_w")[:, 0]
    embw[257:321] = g("emb_flags") @ wf_flags.T
    embw[321] = wf_iat @ g("proj_iat_w")[:, 0]
    embw[322:324] = g("emb_dir") @ wf_dir.T
    embw[324] = (g("fusion_b") + wf_len @ g("proj_len_b")
                 + wf_iat @ g("proj_iat_b"))
    import ml_dtypes
    embw_p = np.zeros((128, 3 * D_MODEL), ml_dtypes.bfloat16)
    for c, (r0, r1) in enumerate(((0, 128), (128, 256), (256, DM_ROWS))):
        embw_p[:r1 - r0, c * D_MODEL:(c + 1) * D_MODEL] = embw[r0:r1]

    A = -np.exp(g("A_log"))           # [L, 512, 16]
    if bool(np.all(A == A[:, :1, :])):
        a_vals = tuple(tuple(float(v) for v in A[l, 0]) for l in range(N_LAYERS))
    else:
        a_vals = None

    wblob = np.zeros((N_LAYERS, 128, WB_COLS), ml_dtypes.bfloat16)
    fblob = np.zeros((N_LAYERS, 128, FB_COLS), np.float32)
    for l in range(N_LAYERS):
        wint = g("in_proj_w")[l].T            # [256, 1024]
        for k in range(2):
            for j in range(8):
                wblob[l, :, WINT + (k * 8 + j) * 128:
                      WINT + (k * 8 + j + 1) * 128] = \
                    wint[k * 128:(k + 1) * 128, j * 128:(j + 1) * 128]
        wdtf = (g("dt_w")[l] @ g("x_proj_w")[l][:DT_RANK, :]).T  # [din, dout]
        for k2 in range(NJ):
            for c in range(NJ):
                wblob[l, :, WDTF + (k2 * 4 + c) * 128:
                      WDTF + (k2 * 4 + c + 1) * 128] = \
                    wdtf[k2 * 128:(k2 + 1) * 128, c * 128:(c + 1) * 128]
        wout = g("out_proj_w")[l].T           # [512, 256]
        for c in range(NJ):
            wblob[l, :, WOUT + c * D_MODEL:WOUT + (c + 1) * D_MODEL] = \
                wout[c * 128:(c + 1) * 128]
        wxbc = g("x_proj_w")[l][DT_RANK:, :].T  # [512, 32]
        for k2 in range(NJ):
            wblob[l, :, WXBC + k2 * 32:WXBC + (k2 + 1) * 32] = \
                wxbc[k2 * 128:(k2 + 1) * 128]
        wblob[l, :, WDP:WDP + NJ] = g("D_param")[l].reshape(NJ, 128).T
        cw_b = np.transpose(g("conv_w")[l].reshape(NJ, 128, D_CONV), (1, 0, 2))
        wblob[l, :, WCW:WCW + 32] = np.repeat(cw_b, B_LOC, axis=1).reshape(128, 32)

        cw = g("conv_w")[l].reshape(NJ, 128, D_CONV)          # [j, p, k]
        cwp = np.transpose(cw, (1, 0, 2))                     # [p, j, k]
        fblob[l, :, 0:32] = np.repeat(cwp, B_LOC, axis=1).reshape(128, 32)
        fblob[l, :, 32:36] = g("conv_b")[l].reshape(NJ, 128).T
        fblob[l, :, 36:40] = g("dt_b")[l].reshape(NJ, 128).T
        Aj = A[l].reshape(NJ, 128, D_STATE)                   # [j, p, n]
        fblob[l, :, 40:104] = np.transpose(Aj, (1, 0, 2)).reshape(128, 64)
        fblob[l, :, 104:108] = g("D_param")[l].reshape(NJ, 128).T

    cblob = np.zeros((128, 260), np.float32)
    w1t = g("cls_w1").T                       # [256, 128]
    cblob[:, 0:128] = w1t[0:128]
    cblob[:, 128:256] = w1t[128:256]
    cblob[:, 256] = g("cls_b1")
    cblob[:, 257:259] = g("cls_w2").T
    cblob[0:2, 259] = g("cls_b2")

    common = {
        "embw": embw_p, "wblob": wblob, "fblob": fblob, "cblob": cblob,
    }

    x = g("x")[:, :T, :]              # causal truncation: only 32 steps matter
    in_maps = []
    for i in range(N_CORES):
        m = dict(common)
        m["x_local"] = np.ascontiguousarray(
            x[i * B_LOC:(i + 1) * B_LOC].reshape(TOK, 5))
        in_maps.append(m)
    return in_maps, a_vals


_PROGRAM_CACHE = {}


def kernel(**inputs) -> np.ndarray:
    in_maps, a_vals = _prep_host(inputs)
    nc = _PROGRAM_CACHE.get(a_vals)
    if nc is None:
        nc = _build_program(a_vals)
        _PROGRAM_CACHE[a_vals] = nc
    res = run_bass_kernel_spmd(nc, in_maps, core_ids=list(range(N_CORES)))
    out = np.zeros((BATCH, 2), np.float32)
    for i in range(N_CORES):
        out[i * B_LOC:(i + 1) * B_LOC] = np.asarray(res.results[i]["out"]).T
    return out
